# revision 4
# baseline (speedup 1.0000x reference)
"""Attentional pooling v2 — structural perf changes over the baseline:

  1. Score-path matmuls/transposes in f32r (1 / 1.5 cyc/row vs 4 / 2 fp32).
  2. PE-path scores transposed to columns BEFORE exp: ACT exp reads PSUM
     [128,2] + SBUF [128,6] with accum_out; no [1,256]-row exp, no e_ps
     copies.
  3. B loaded in 2 MiB DMAs (two 8-tile groups per dma_start) -> half the
     SWDGE issues and sem waits on Pool.

Math per sample b:
    AP  = tanh(A @ W.T + b)                      [256]
    R_t = B[b, t, :] . AP                        [8192]
    e_t = exp(-R_t)
    out = (sum_t e_t * B[b, t, :]) / (sum_t e_t) [256]

Sharding: batch 64 -> 8 cores x 8 samples, weights replicated.
"""
import numpy as np
from contextlib import ExitStack

import concourse.bacc as bacc
import concourse.tile as tile
import concourse.mybir as mybir
from concourse.bass_utils import run_bass_kernel_spmd
from concourse.masks import make_identity

F32 = mybir.dt.float32
F32R = mybir.dt.float32r
EXP = mybir.ActivationFunctionType.Exp
TANH = mybir.ActivationFunctionType.Tanh
MUL = mybir.AluOpType.mult
ADD = mybir.AluOpType.add

BATCH, T, FEAT, HID = 64, 8192, 512, 256
NCORES = 8
SPC = BATCH // NCORES          # samples per core
NT = T // 128                  # 64 t-tiles per sample
GRP = 8                        # t-tiles per compute group
NG = NT // GRP                 # 8 compute groups per sample
NPAIR = NG // 2                # 4 DMA pairs (2 MiB each) per sample


def build_nc(reps: int = 1, dma_only: bool = False):
    nc = bacc.Bacc("TRN2", target_bir_lowering=False, debug=False)

    A = nc.dram_tensor("A", [SPC, FEAT], F32, kind="ExternalInput").ap()
    B = nc.dram_tensor("B", [SPC, T, HID], F32R, kind="ExternalInput").ap()
    W = nc.dram_tensor("W", [HID, FEAT], F32, kind="ExternalInput").ap()
    BIAS = nc.dram_tensor("BIAS", [HID], F32, kind="ExternalInput").ap()
    OUT = nc.dram_tensor("OUT", [SPC, HID], F32, kind="ExternalOutput").ap()

    # DMA view: one pair = 2 MiB, FULLY CONTIGUOUS per partition line
    # (16 KiB = 16 consecutive t-rows per partition). The t <-> (p, u)
    # relabeling is free: scores/exp/pooling are permutation-invariant in t.
    Bv = B.rearrange("s (q p u) h -> s q p (u h)", q=NPAIR, p=128, u=2 * GRP)
    Wv = W.rearrange("(c p) f -> c p f", p=128)
    BIASv = BIAS.rearrange("(a h) -> a h", a=1)

    with tile.TileContext(nc) as tc, ExitStack() as ctx:
        cst = ctx.enter_context(tc.tile_pool(name="cst", bufs=1))
        ps_scr = ctx.enter_context(tc.tile_pool(name="ps_scr", bufs=1, space="PSUM"))

        ident = cst.tile([128, 128], F32, tag="ident")
        make_identity(nc, ident[:])
        ident_r = cst.tile([128, 128], F32R, tag="ident_r")
        nc.vector.tensor_copy(ident_r[:], ident[:])
        zbias = cst.tile([128, 1], F32, tag="zbias")
        nc.gpsimd.memset(zbias[:], 0.0)
        ones_c = cst.tile([128, 1], F32, tag="ones_c")
        nc.gpsimd.memset(ones_c[:], 1.0)
        ones_row = cst.tile([1, 128], F32, tag="ones_row")
        nc.gpsimd.memset(ones_row[:], 1.0)
        act_scr = cst.tile([1, 1], F32, tag="act_scr")
        pool_scr = cst.tile([1, 1], F32, tag="pool_scr")
        dve_mark = cst.tile([1, 1], F32, tag="dve_mark")
        nc.gpsimd.memset(dve_mark[:], 0.0)
        # sel_all[k, s*128 + m] = 1 if k == s else 0 (per-sample broadcast selector)
        sel_all = cst.tile([SPC, SPC * 128], F32, tag="sel_all")
        nc.gpsimd.memset(sel_all[:], 0.0)
        nc.gpsimd.affine_select(
            out=sel_all[:].rearrange("k (s m) -> k s m", s=SPC),
            in_=sel_all[:].rearrange("k (s m) -> k s m", s=SPC),
            compare_op=mybir.AluOpType.not_equal,
            fill=1.0,
            base=0,
            pattern=[[-1, SPC], [0, 128]],
            channel_multiplier=1,
        )

        w_sb = cst.tile([128, 2 * FEAT], F32, tag="w_sb")
        nc.gpsimd.dma_start(w_sb[:, 0:FEAT], Wv[0])
        nc.gpsimd.dma_start(w_sb[:, FEAT : 2 * FEAT], Wv[1])
        a_sb = cst.tile([SPC, FEAT], F32, tag="a_sb")
        nc.gpsimd.dma_start(a_sb[:], A)
        b_row = cst.tile([1, HID], F32, tag="b_row")
        nc.gpsimd.dma_start(b_row[:], BIASv)

        scr = ps_scr.tile([1, 1], F32, tag="scr")

        def absorb(ap):
            nc.tensor.transpose(scr[:], ap, ident[0:1, 0:1])

        # ACT absorber for gpsimd-made constants (also loads the exp table set)
        nc.scalar.activation(act_scr[:], zbias[0:1, 0:1], EXP, bias=zbias[0:1, 0:1])
        # PE absorbers: gpsimd consts + the input DMA queues
        absorb(ident[0:1, 0:1])
        absorb(w_sb[0:1, 0:1])
        absorb(w_sb[0:1, FEAT : FEAT + 1])
        absorb(a_sb[0:1, 0:1])
        absorb(b_row[0:1, 0:1])

        # ---- AP = tanh(A @ W.T + bias) ----
        with tc.tile_pool(name="setup_ps", bufs=1, space="PSUM") as sps, \
             tc.tile_pool(name="setup_sb", bufs=1) as ssb:
            wt_sb = ssb.tile([128, 4 * HID], F32, tag="wt_sb")
            for fc in range(4):
                wt_ps = sps.tile([128, HID], F32, tag="wt_ps", name=f"wt_ps{fc}", bufs=2)
                for c in range(2):
                    nc.tensor.transpose(
                        wt_ps[:, c * 128 : (c + 1) * 128],
                        w_sb[:, c * FEAT + fc * 128 : c * FEAT + (fc + 1) * 128],
                        ident[:],
                    )
                nc.vector.tensor_copy(wt_sb[:, fc * HID : (fc + 1) * HID], wt_ps[:])

            at_ps = sps.tile([128, 4 * SPC], F32, tag="at_ps")
            for fc in range(4):
                nc.tensor.transpose(
                    at_ps[:, fc * SPC : (fc + 1) * SPC],
                    a_sb[0:SPC, fc * 128 : (fc + 1) * 128],
                    ident[0:SPC, 0:SPC],
                )
            at_sb = ssb.tile([128, 4 * SPC], F32, tag="at_sb")
            nc.vector.tensor_copy(at_sb[:], at_ps[:])

            ap_ps = sps.tile([SPC, HID], F32, tag="ap_ps")
            for fc in range(4):
                nc.tensor.matmul(
                    ap_ps[:],
                    at_sb[:, fc * SPC : (fc + 1) * SPC],
                    wt_sb[:, fc * HID : (fc + 1) * HID],
                    start=(fc == 0),
                    stop=False,
                )
            nc.tensor.matmul(ap_ps[:], ones_row[:, 0:SPC], b_row[:], start=False, stop=True)

            ap_sb = cst.tile([SPC, HID], F32, tag="ap_sb")
            nc.scalar.activation(ap_sb[:], ap_ps[:], TANH, bias=zbias[0:SPC])

            # broadcast every AP row to 128 partitions now: sel_s.T @ ap_sb
            apb_all = cst.tile([128, SPC * HID], F32, tag="apb_all")
            for _s in range(SPC):
                apb_ps = sps.tile([128, HID], F32, tag="apb_ps", name=f"apb_ps{_s}", bufs=2)
                nc.tensor.matmul(
                    apb_ps[:],
                    sel_all[0:SPC, _s * 128 : (_s + 1) * 128],
                    ap_sb[:],
                )
                nc.vector.tensor_copy(apb_all[:, _s * HID : (_s + 1) * HID], apb_ps[:])

            # AP transposed to [h, s] for the PE score path (f32r operand)
            apt_ps = sps.tile([128, 2 * SPC], F32, tag="apt_ps")
            for hc in range(2):
                nc.tensor.transpose(
                    apt_ps[:, hc * SPC : (hc + 1) * SPC],
                    ap_sb[0:SPC, hc * 128 : (hc + 1) * 128],
                    ident[0:SPC, 0:SPC],
                )
            apt_sb = cst.tile([128, 2 * SPC], F32R, tag="apt_sb")
            nc.vector.tensor_copy(apt_sb[:], apt_ps[:])

        # ---- main pools ----
        bpool = ctx.enter_context(tc.tile_pool(name="bpool", bufs=4))
        rpool = ctx.enter_context(tc.tile_pool(name="rpool", bufs=2))
        erpool = ctx.enter_context(tc.tile_pool(name="erpool", bufs=NG))
        small = ctx.enter_context(tc.tile_pool(name="small", bufs=2))
        con_pool = ctx.enter_context(tc.tile_pool(name="con", bufs=1))
        junk_pool = ctx.enter_context(tc.tile_pool(name="junk", bufs=1))
        g_pool = ctx.enter_context(tc.tile_pool(name="g_ps", bufs=2, space="PSUM"))
        tp_pool = ctx.enter_context(tc.tile_pool(name="tp_ps", bufs=2, space="PSUM"))
        rcol_pool = ctx.enter_context(tc.tile_pool(name="rcol_ps", bufs=2, space="PSUM"))
        btg_pool = ctx.enter_context(tc.tile_pool(name="btg", bufs=2))

        con_wide = con_pool.tile([1, SPC * HID], F32, tag="con_wide")
        # one junk range per bpool slot: the pre-DMA absorb then only waits
        # for the DVE work that read the slot being overwritten
        NBUF = 4
        junk = junk_pool.tile([128, NBUF * HID], F32, tag="junk")

        started = False
        pending_tail = None
        for rep in range(reps):
            for s in range(SPC):
                if started and not dma_only:
                    prev = (s + SPC - 1) % SPC
                    absorb(con_wide[0:1, prev * HID : prev * HID + 1])

                ap_bcast = apb_all[:, s * HID : (s + 1) * HID]

                dacc = small.tile([128, 2 * NG], F32, tag="dacc")
                G = g_pool.tile([1, HID + 8], F32, tag="G")

                prev_grp = None  # (e_r, bgrp_half, g) of previous compute group
                for g in range(NG):
                    q, half = divmod(g, 2)
                    qslot = (rep * SPC * NPAIR + s * NPAIR + q) % NBUF
                    jnk = junk[:, qslot * HID : (qslot + 1) * HID]
                    if g == 1 and pending_tail is not None:
                        pending_tail()
                        pending_tail = None
                    if half == 0:
                        if (started or g > 0) and not dma_only:
                            # absorb DVE tick on Pool so the B load carries
                            # only the PE (pooling WAR) wait; per-slot junk
                            # range keeps this wait short
                            nc.gpsimd.tensor_copy(pool_scr[:], jnk[0:1, 0:1])
                        bpair = bpool.tile([128, 2 * GRP * HID], F32R, tag="bpair")
                        nc.gpsimd.dma_start(bpair[:], Bv[s, q])

                    if dma_only:
                        prev_grp = None
                        continue
                    bgrp = bpair[:, half * GRP * HID : (half + 1) * GRP * HID]
                    e_r = erpool.tile([128, GRP], F32R, tag="e_r")

                    # --- PE score path for tiles j=0,1 (all f32r) ---
                    tp_ps = tp_pool.tile([128, 512], F32, tag="tp_ps")
                    for c in range(2):
                        for j in range(2):
                            nc.tensor.transpose(
                                tp_ps[:, (c * 2 + j) * 128 : (c * 2 + j + 1) * 128].bitcast(F32R),
                                bgrp[:, j * HID + c * 128 : j * HID + (c + 1) * 128],
                                ident_r[:],
                            )
                    btg = btg_pool.tile([128, 512], F32R, tag="btg")
                    nc.scalar.copy(btg[:], tp_ps[:])
                    # score COLUMNS directly: r_all[:, j*SPC + k] = btg(:,j).T @ apt_k
                    # (all SPC samples as moving columns; only column s is used —
                    #  keeps the f32r moving free-dim a multiple of 4)
                    r_ps = rcol_pool.tile([128, 2 * SPC], F32, tag="r_ps")
                    for j in range(2):
                        for c in range(2):
                            nc.tensor.matmul(
                                r_ps[:, j * SPC : (j + 1) * SPC],
                                btg[:, (c * 2 + j) * 128 : (c * 2 + j + 1) * 128],
                                apt_sb[:, c * SPC : (c + 1) * SPC],
                                start=(c == 0),
                                stop=(c == 1),
                                skip_group_check=True,
                            )
                    r_mine = r_ps[:].rearrange("p (j k) -> p j k", j=2)[:, :, s]
                    nc.scalar.activation(
                        e_r[:, 0:2], r_mine, EXP, scale=-1.0, bias=zbias[:],
                        accum_out=dacc[:, 2 * g : 2 * g + 1],
                    )

                    # --- DVE score path for tiles j=2..7 ---
                    r_sb = rpool.tile([128, GRP], F32, tag="r_sb")
                    for j in range(2, GRP):
                        if j == 4 and prev_grp is not None:
                            per_, pbgrp, pg = prev_grp
                            for jj in range(GRP):
                                nc.tensor.matmul(
                                    G[0:1, 0:HID],
                                    per_[:, jj : jj + 1],
                                    pbgrp[:, jj * HID : (jj + 1) * HID],
                                    start=(pg == 0 and jj == 0),
                                    stop=False,
                                    skip_group_check=True,
                                )
                        nc.vector.scalar_tensor_tensor(
                            out=jnk,
                            in0=bgrp[:, j * HID : (j + 1) * HID].bitcast(F32),
                            scalar=1.0,
                            in1=ap_bcast[:],
                            op0=MUL,
                            op1=MUL,
                            accum_out=r_sb[:, j : j + 1],
                        )

                    nc.scalar.activation(
                        e_r[:, 2:GRP], r_sb[:, 2:GRP], EXP, scale=-1.0, bias=zbias[:],
                        accum_out=dacc[:, 2 * g + 1 : 2 * g + 2],
                    )
                    prev_grp = (e_r, bgrp, g)
                    started = True

                # flush last group's pooling
                if dma_only:
                    continue
                per_, pbgrp, pg = prev_grp
                for jj in range(GRP):
                    nc.tensor.matmul(
                        G[0:1, 0:HID],
                        per_[:, jj : jj + 1],
                        pbgrp[:, jj * HID : (jj + 1) * HID],
                        start=False,
                        stop=False,
                        skip_group_check=True,
                    )

                # den/scale tail: defer into the next sample's stream
                def make_tail(dacc=dacc, G=G, s=s):
                    def tail():
                        dsum = small.tile([128, 1], F32, tag="dsum", name="dsum")
                        nc.vector.tensor_reduce(
                            dsum[:], dacc[:], axis=mybir.AxisListType.X, op=ADD
                        )
                        nc.tensor.matmul(
                            G[0:1, HID : HID + 1], ones_c[:], dsum[:],
                            start=False, stop=True, skip_group_check=True,
                        )
                        inv = small.tile([1, 1], F32, tag="inv", name="inv")
                        nc.vector.reciprocal(inv[:], G[0:1, HID : HID + 1])
                        nc.vector.tensor_scalar_mul(
                            con_wide[0:1, s * HID : (s + 1) * HID], G[0:1, 0:HID], inv[:]
                        )
                        nc.vector.tensor_copy(dve_mark[:], inv[:])
                    return tail
                pending_tail = make_tail()

        if pending_tail is not None:
            pending_tail()
            pending_tail = None
        if dma_only:
            nc.vector.tensor_copy(con_wide[0:1, 0:1], zbias[0:1, 0:1])
        OUTv = OUT.rearrange("s h -> (s h)").rearrange("(a n) -> a n", a=1)
        nc.gpsimd.dma_start(OUTv, con_wide[0:1, :])

    nc.compile()
    return nc


_NC_CACHE = {}


def _get_nc(reps: int = 1):
    if reps not in _NC_CACHE:
        _NC_CACHE[reps] = build_nc(reps)
    return _NC_CACHE[reps]


def kernel(A, B, W, b):
    A = np.asarray(A, dtype=np.float32)
    B = np.asarray(B, dtype=np.float32)
    W = np.asarray(W, dtype=np.float32)
    b = np.asarray(b, dtype=np.float32)

    nc = _get_nc(1)
    in_maps = [
        {
            "A": A[c * SPC : (c + 1) * SPC],
            "B": B[c * SPC : (c + 1) * SPC],
            "W": W,
            "BIAS": b,
        }
        for c in range(NCORES)
    ]
    res = run_bass_kernel_spmd(nc, in_maps, list(range(NCORES))).results
    out = np.concatenate([r["OUT"] for r in res], axis=0)
    return out[:, None, :].astype(np.float32)
